# revision 29
# baseline (speedup 1.0000x reference)
"""BiMPMMatching Trainium2 Bass kernel.

Pure data parallel: batch (B=8) sharded one element per NeuronCore.
Each core computes the full BiMPM matching for its (S1=256, S2=256, H=100,
P=20) element and writes a (2, 256, 105) output; host stacks to
(2, 8, 256, 105).

Decomposition highlights (validated vs reference in fp32 to ~8e-5 rel):
  - cosine matrices via PE matmuls of pre-normalized operands, with an extra
    "ones" row on lhsT and an "offset" row ((1-mask)*MIN_VAL) on rhs so the
    masked-max exclusion rides along in the matmul output, plus an extra
    rhs column holding rowsums (serves masked-mean and attention denom).
  - maxpool-match: per-perspective matmuls with the reduced-side norm
    (rn2) folded into rhs (via DMA partition-broadcast of the rn row) and
    the kept-side norm applied after the reduction (max is positively
    homogeneous).  Means come from a single small G-matmul per side.
  - max-attentive: att_max[i,h] = max_j (att+off)[i,j] * chb[j,h] where
    chb is mask-replaced (invalid rows := 1.0) so invalid j contribute
    -1e7.  Computed with fused tensor_tensor_reduce ops (mult+max) in
    bf16 against a partition-broadcast replica of chb.
"""

import os
import numpy as np

import concourse.bass as bass
import concourse.mybir as mybir
import concourse.tile as tile
from concourse.bass_utils import run_bass_kernel_spmd
from concourse.masks import make_identity

F32 = mybir.dt.float32
BF16 = mybir.dt.bfloat16
I32 = mybir.dt.int32
AF = mybir.ActivationFunctionType
OP = mybir.AluOpType
AX = mybir.AxisListType

S = 256   # S1 == S2
H = 100
P = 20
NW = 80   # 4*P stacked perspectives
MIN_VAL = -1e7
EPS = 1e-8
CL = 1e-12  # norm^2 clamp added under sqrt
N_CORES = 8

# rn table rows: 0 = plain norm, 1..80 = perspectives [full, maxpool, att, ma]
GRP_FULL = 1
GRP_MP = 21
GRP_ATT = 41
GRP_MA = 61


def _split_multi_waits(nc):
    """This walrus build only encodes one sync wait (and one update) per
    instruction; Tile emits several.  Split extras into standalone
    EventSemaphore ops on the same engine (engine stream order preserves
    semantics)."""
    for f in nc.m.functions:
        for blk in f.blocks:
            out = []
            for inst in blk.instructions:
                si = inst.sync_info
                if si is not None and len(si.on_wait) > 1:
                    waits = list(si.on_wait)
                    for w in waits[:-1]:
                        ev = mybir.InstEventSemaphore(
                            name=nc.get_next_instruction_name(),
                            engine=inst.engine, ins=[], outs=[],
                            sync_info=mybir.SyncInfo(on_wait=[w],
                                                     on_update=[]))
                        nc.register_instruction(ev)
                        out.append(ev)
                    si.on_wait = [waits[-1]]
                post = []
                if si is not None and len(si.on_update) > 1:
                    assert type(inst).__name__ != "InstDMACopy", (
                        "can't move a DMA completion update")
                    ups = list(si.on_update)
                    si.on_update = [ups[0]]
                    for u in ups[1:]:
                        ev = mybir.InstEventSemaphore(
                            name=nc.get_next_instruction_name(),
                            engine=inst.engine, ins=[], outs=[],
                            sync_info=mybir.SyncInfo(on_wait=[],
                                                     on_update=[u]))
                        nc.register_instruction(ev)
                        post.append(ev)
                out.append(inst)
                out.extend(post)
            blk.instructions[:] = out


def _build(nc):
    # ---------------- DRAM I/O ----------------
    d_cpT = nc.dram_tensor("cpT", [H, S], F32, kind="ExternalInput")
    d_chT = nc.dram_tensor("chT", [H, S], F32, kind="ExternalInput")
    d_cps = nc.dram_tensor("cps", [S, H], F32, kind="ExternalInput")
    d_chs = nc.dram_tensor("chs", [S, H], F32, kind="ExternalInput")
    d_mp = nc.dram_tensor("mp", [1, S], I32, kind="ExternalInput")
    d_mh = nc.dram_tensor("mh", [1, S], I32, kind="ExternalInput")
    d_wT = nc.dram_tensor("wT", [H, NW], F32, kind="ExternalInput")
    d_out = nc.dram_tensor("out", [2, S, 105], F32, kind="ExternalOutput")

    with tile.TileContext(nc) as tc:
        _emit(nc, tc, d_cpT, d_chT, d_cps, d_chs, d_mp, d_mh, d_wT, d_out)
    _split_multi_waits(nc)
    return nc


def _emit(nc, tc, d_cpT, d_chT, d_cps, d_chs, d_mp, d_mh, d_wT, d_out):
    from contextlib import ExitStack
    ablate = set(os.environ.get("KABLATE", "").split(","))
    ctx = ExitStack()
    persist = ctx.enter_context(tc.tile_pool(name="persist", bufs=1))
    work = ctx.enter_context(tc.tile_pool(name="work", bufs=3))
    ps_pool = ctx.enter_context(tc.tile_pool(name="ps", bufs=3, space="PSUM"))
    dram = ctx.enter_context(tc.tile_pool(name="dram", bufs=1, space="DRAM"))

    dma = nc.sync.dma_start
    v = nc.vector
    sc = nc.scalar
    gs = nc.gpsimd

    # ---------------- constants ----------------
    padn = int(os.environ.get("KPAD", "0"))
    if padn:
        pad_t = persist.tile([128, padn], F32, tag="pad")
        v_pad = nc.vector
        v_pad.memset(pad_t, 0.0)
    ident = persist.tile([128, 128], F32, tag="ident")
    make_identity(nc, ident)
    ones_row = persist.tile([1, 128], F32, tag="ones_row")
    v.memset(ones_row, 1.0)
    cl_col = persist.tile([128, 1], F32, tag="cl_col")
    v.memset(cl_col, CL)

    # ---------------- load weights, build lhs_n = [ones | W^2] (H, 81) -----
    wT_sb = work.tile([H, NW], F32, tag="wT")
    dma(out=wT_sb, in_=d_wT[:])
    lhs_n = persist.tile([H, 1 + NW], F32, tag="lhs_n")
    v.memset(lhs_n[:, 0:1], 1.0)
    v.tensor_mul(lhs_n[:, 1:1 + NW], wT_sb, wT_sb)

    # G bases for att / ma groups (ones col + group cols)
    g_att = persist.tile([H, 21], F32, tag="g_att")
    v.tensor_copy(g_att[:, 0:1], lhs_n[:, 0:1])
    v.tensor_copy(g_att[:, 1:21], lhs_n[:, GRP_ATT:GRP_ATT + 20])
    g_ma = persist.tile([H, 21], F32, tag="g_ma")
    v.tensor_copy(g_ma[:, 0:1], lhs_n[:, 0:1])
    v.tensor_copy(g_ma[:, 1:21], lhs_n[:, GRP_MA:GRP_MA + 20])

    # ---------------- per-side precompute ----------------
    sides = {}
    for name, d_xT, d_xs, d_m in (("p", d_cpT, d_cps, d_mp),
                                  ("h", d_chT, d_chs, d_mh)):
        sd = {}
        # mask broadcast (128, S) int32 -> f32
        m_b_i = work.tile([128, 1, S], I32, tag="m_b_i")
        dma(out=m_b_i, in_=d_m[:].partition_broadcast(128))
        m_b = persist.tile([128, S], F32, tag=f"m_b_{name}")
        v.tensor_copy(m_b, m_b_i[:, 0, :])
        # mask as column (128, 2, 1)
        m_col_i = work.tile([128, 2, 1], I32, tag="m_col_i")
        dma(out=m_col_i, in_=d_m[0, :].rearrange("(t p) -> p t", p=128))
        m_col = persist.tile([128, 2, 1], F32, tag=f"m_col_{name}")
        v.tensor_copy(m_col, m_col_i)
        # off row: (1-m)*MIN_VAL = m*(-MIN_VAL) + MIN_VAL
        off_row = persist.tile([1, S], F32, tag=f"off_{name}")
        sc.activation(off_row, m_b[0:1, :], AF.Copy, bias=MIN_VAL,
                      scale=-MIN_VAL)
        # len / invlen
        len_t = persist.tile([1, 1], F32, tag=f"len_{name}")
        v.reduce_sum(len_t, m_b[0:1, :], axis=AX.X)
        invlen = persist.tile([1, 1], F32, tag=f"invlen_{name}")
        v.reciprocal(invlen, len_t)
        ps_il = ps_pool.tile([128, 512], F32, tag="ps_a")
        nc.tensor.matmul(ps_il[:, 0:1], ones_row, invlen, start=True,
                         stop=True)
        invlen_col = persist.tile([128, 1], F32, tag=f"invlen_col_{name}")
        sc.activation(invlen_col, ps_il[:, 0:1], AF.Copy, bias=0.0, scale=1.0)

        # masked T layout with ones row: (101, S).  Engine APs must start at
        # partition 0/32/64/96, so fill rows 96:101 first, then overwrite
        # the data rows 0:100.
        xTm = persist.tile([101, S], F32, tag=f"xTm_{name}")
        xT_sb = work.tile([H, S], F32, tag="xT_in")
        dma(out=xT_sb, in_=d_xT[:])
        v.memset(xTm[96:101, :], 1.0)
        v.tensor_mul(xTm[0:H, :], xT_sb, m_b[0:H, :])

        # masked S layout (128, 2, H) + bf16 copy
        xs_sb = work.tile([128, 2, H], F32, tag="xs_in")
        dma(out=xs_sb, in_=d_xs[:].rearrange("(t p) h -> p t h", p=128))
        xm_s = persist.tile([128, 2, H], F32, tag=f"xm_s_{name}")
        for t in range(2):
            v.tensor_scalar_mul(xm_s[:, t, :], xs_sb[:, t, :], m_col[:, t, :])
        xm_s16 = persist.tile([128, 2, H], BF16, tag=f"xm_s16_{name}")
        sc.activation(xm_s16.rearrange("p a b -> p (a b)"),
                      xm_s.rearrange("p a b -> p (a b)"), AF.Copy,
                      bias=0.0, scale=1.0)

        # norms: nsq (81, S) = lhs_n.T @ xTm^2 ; rn = 1/sqrt(nsq + CL)
        sqT = work.tile([H, S], F32, tag="sqT")
        sc.square(sqT, xTm[0:H, :])
        ps_n = ps_pool.tile([128, 512], F32, tag="ps_a")
        nc.tensor.matmul(ps_n[0:81, 0:S], lhs_n[:, 0:81], sqT, start=True,
                         stop=True)
        n_sb = work.tile([81, S], F32, tag="n_sb")
        sc.activation(n_sb, ps_n[0:81, 0:S], AF.Sqrt, bias=cl_col[0:81],
                      scale=1.0)
        rn = persist.tile([81, S], F32, tag=f"rn_{name}")
        v.reciprocal(rn, n_sb)
        # rnT (128, 2, 81)
        rnT = persist.tile([128, 2, 81], F32, tag=f"rnT_{name}")
        for t in range(2):
            ps_t = ps_pool.tile([128, 512], F32, tag="ps_b", bufs=4)
            nc.tensor.transpose(ps_t[:, 0:81], rn[:, t * 128:(t + 1) * 128],
                                ident[0:81, 0:81])
            sc.activation(rnT[:, t, :], ps_t[:, 0:81], AF.Copy, bias=0.0,
                          scale=1.0)
        # stage rn to DRAM (bf16) for row-broadcasts
        rn16 = work.tile([81, S], BF16, tag="rn16")
        sc.activation(rn16, rn, AF.Copy, bias=0.0, scale=1.0)
        d_rn = dram.tile([81, S], BF16, tag=f"d_rn_{name}")
        dma(out=d_rn[:], in_=rn16)

        # normalized lhsT [Nhat; ones] (101, S) and rhs [Nhat; off | sums]
        ps_r0 = ps_pool.tile([128, 512], F32, tag="ps_a")
        nc.tensor.matmul(ps_r0[:, 0:S], ones_row, rn[0:1, :], start=True,
                         stop=True)
        nt_lhs = persist.tile([101, S], F32, tag=f"nt_lhs_{name}")
        v.memset(nt_lhs[96:101, :], 1.0)
        v.tensor_mul(nt_lhs[0:H, :], xTm[0:H, :], ps_r0[0:H, 0:S])
        nt_rhs = persist.tile([101, S + 1], F32, tag=f"nt_rhs_{name}")
        sc.activation(nt_rhs[96:101, 0:S], m_b[96:101, :], AF.Copy,
                      bias=MIN_VAL, scale=-MIN_VAL)
        v.memset(nt_rhs[96:101, S:S + 1], 0.0)
        sc.activation(nt_rhs[0:H, 0:S], nt_lhs[0:H, :], AF.Copy,
                      bias=0.0, scale=1.0,
                      accum_out=nt_rhs[0:H, S:S + 1])

        # mask-replaced T-layout for products: xTm + (1 - m)  -> bf16 -> DRAM
        rep_b = work.tile([128, S], F32, tag="rep_b")
        sc.activation(rep_b, m_b, AF.Copy, bias=1.0, scale=-1.0)
        xrep = work.tile([H, S], F32, tag="xrep")
        v.tensor_add(xrep, xTm[0:H, :], rep_b[0:H, :])
        xrep16 = work.tile([H, S], BF16, tag="xrep16")
        sc.activation(xrep16, xrep, AF.Copy, bias=0.0, scale=1.0)
        d_rep = dram.tile([H, S], BF16, tag=f"d_rep_{name}")
        dma(out=d_rep[:], in_=xrep16)

        # one-hot (last valid) column (128, 2, 1)
        ohe = work.tile([1, S + 1], F32, tag="ohe")
        v.tensor_copy(ohe[:, 0:S], m_b[0:1, :])
        v.memset(ohe[:, S:S + 1], 0.0)
        oh_row = work.tile([1, S], F32, tag="oh_row")
        v.tensor_tensor(oh_row, ohe[:, 0:S], ohe[:, 1:S + 1], op=OP.subtract)
        oh_col = persist.tile([128, 2, 1], F32, tag=f"oh_col_{name}")
        for t in range(2):
            ps_oh = ps_pool.tile([128, 512], F32, tag="ps_b", bufs=4)
            nc.tensor.transpose(ps_oh[:, 0:1],
                                oh_row[0:1, t * 128:(t + 1) * 128],
                                ident[0:1, 0:1])
            sc.activation(oh_col[:, t, :], ps_oh[:, 0:1], AF.Copy, bias=0.0,
                          scale=1.0)

        # bf16 copy of masked-T layout for the mp-max matmul lhsT
        xTm16 = persist.tile([101, S], BF16, tag=f"xTm16_{name}")
        sc.activation(xTm16, xTm, AF.Copy, bias=0.0, scale=1.0)
        sd["xTm16"] = xTm16
        sd.update(m_b=m_b, m_col=m_col, off_row=off_row, invlen=invlen,
                  invlen_col=invlen_col, xTm=xTm, xm_s=xm_s, xm_s16=xm_s16,
                  rn=rn, rnT=rnT, d_rn=d_rn, nt_lhs=nt_lhs, nt_rhs=nt_rhs,
                  d_rep=d_rep, oh_col=oh_col)

        # comb tiles (128, 2, 21): [rn0 | group rows] transposed
        for gname, g0 in (("full", GRP_FULL), ("att", GRP_ATT),
                          ("ma", GRP_MA)):
            comb = persist.tile([128, 2, 21], F32, tag=f"comb_{gname}_{name}")
            sc.activation(comb[:, :, 0:1], rnT[:, :, 0:1], AF.Copy,
                          bias=0.0, scale=1.0)
            sc.activation(comb[:, :, 1:21], rnT[:, :, g0:g0 + 20], AF.Copy,
                          bias=0.0, scale=1.0)
            sd[f"comb_{gname}"] = comb
        sides[name] = sd

    # chunked partition-broadcast replicas of the replaced contexts (bf16);
    # chunk tiles are shared between the two directions (sequential reuse)
    HC = 25
    NCH = H // HC
    bc_pool = ctx.enter_context(tc.tile_pool(name="bc", bufs=1))
    tree_pool = ctx.enter_context(tc.tile_pool(name="tree", bufs=2))
    mp_pool = ctx.enter_context(tc.tile_pool(name="mp", bufs=2))

    # out staging
    out_sb = {name: persist.tile([128, 2, 105], F32, tag=f"out_{name}",
                                 name=f"out_{name}")
              for name in ("p", "h")}

    # ---------------- cos matmuls + att evac, per direction ----------------
    att_sb = {}
    for d, (A, B) in enumerate((("p", "h"), ("h", "p"))):
        sa, sb = sides[A], sides[B]
        a_sb = persist.tile([128, 2, 258], BF16, tag=f"att_sb_{A}")
        for t in range(2):
            ps_att = ps_pool.tile([128, 512], F32, tag="ps_a")
            nc.tensor.matmul(ps_att[:, 0:S + 1],
                             sa["nt_lhs"][:, t * 128:(t + 1) * 128],
                             sb["nt_rhs"][:],
                             start=True, stop=True)
            # evac att(+off) in bf16 (scalar engine: PSUM -> SBUF cast)
            sc.activation(a_sb[:, t, 0:S + 1], ps_att[:, 0:S + 1], AF.Copy,
                          bias=0.0, scale=1.0)
            # cos_max / cos_mean
            v.reduce_max(out_sb[A][:, t, 0:1], ps_att[:, 0:S], axis=AX.X)
            sc.activation(out_sb[A][:, t, 1:2], ps_att[:, S:S + 1], AF.Copy,
                          bias=0.0, scale=sb["invlen_col"])
        att_sb[A] = a_sb

    # ---------------- early small sections (deps ready now) --------------
    # mv-att: meanT[h,i] = sum_j att[i,j]*chm[j,h]; the 1/max(sum,EPS)
    # normalizer is a positive per-i scale that cancels in every cosine it
    # feeds, so it is never computed.  cm/m2T feed G-matmuls in (h-part, i)
    # layout — no PE transposes.
    for A, B in (("p", "h"), ("h", "p")):
        sa, sb = sides[A], sides[B]
        ps_mT = ps_pool.tile([128, 512], F32, tag="ps_b", bufs=4)
        for jt in range(2):
            nc.tensor.matmul(ps_mT[0:H, 0:S], sb["xm_s16"][:, jt, :],
                             att_sb[B][:, jt, 0:S],
                             start=(jt == 0), stop=(jt == 1))
        cm = work.tile([H, S], F32, tag="cm_att", name=f"cm_att_{A}")
        v.tensor_mul(cm, sa["xTm"][0:H, :], ps_mT[0:H, 0:S])
        m2T = work.tile([H, S], F32, tag="m2_att", name=f"m2_att_{A}")
        sc.square(m2T, ps_mT[0:H, 0:S])
        for t in range(2):
            ps_num = ps_pool.tile([128, 512], F32, tag="ps_b", bufs=4)
            nc.tensor.matmul(ps_num[:, 0:21],
                             cm[:, t * 128:(t + 1) * 128], g_att,
                             start=True, stop=True)
            ps_msq = ps_pool.tile([128, 512], F32, tag="ps_b", bufs=4)
            nc.tensor.matmul(ps_msq[:, 0:21],
                             m2T[:, t * 128:(t + 1) * 128], g_att,
                             start=True, stop=True)
            nm = work.tile([128, 21], F32, tag="nm")
            sc.activation(nm, ps_msq[:, 0:21], AF.Sqrt, bias=cl_col,
                          scale=1.0)
            rnm = work.tile([128, 21], F32, tag="rnm")
            v.reciprocal(rnm, nm)
            t21 = work.tile([128, 21], F32, tag="t21")
            v.tensor_tensor(t21, rnm, sa["comb_att"][:, t, :], op=OP.mult)
            v.tensor_tensor(out_sb[A][:, t, 63:84], ps_num[:, 0:21],
                            t21, op=OP.mult)

    # ---------------- full match ----------------
    for A, B in (("p", "h"), ("h", "p")):
        sa, sb = sides[A], sides[B]
        ps_lh = ps_pool.tile([128, 512], F32, tag="ps_b", bufs=4)
        for jt in range(2):
            nc.tensor.matmul(ps_lh[0:H, 0:1], sb["xm_s"][:, jt, :],
                             sb["oh_col"][:, jt, :],
                             start=(jt == 0), stop=(jt == 1))
        lh_sb = work.tile([H, 1], F32, tag="lh_sb")
        sc.activation(lh_sb, ps_lh[0:H, 0:1], AF.Copy, bias=0.0, scale=1.0)
        lhsq = work.tile([H, 1], F32, tag="lhsq")
        sc.square(lhsq, lh_sb)
        ps_nl = ps_pool.tile([128, 512], F32, tag="ps_b", bufs=4)
        nc.tensor.matmul(ps_nl[0:1, 0:81], lhsq, lhs_n[:, 0:81], start=True,
                         stop=True)
        nl_sb = work.tile([1, 81], F32, tag="nl_sb")
        sc.activation(nl_sb, ps_nl[0:1, 0:81], AF.Sqrt, bias=cl_col[0:1],
                      scale=1.0)
        rnl = work.tile([1, 81], F32, tag="rnl")
        v.reciprocal(rnl, nl_sb)
        ps_rb = ps_pool.tile([128, 512], F32, tag="ps_b", bufs=4)
        nc.tensor.matmul(ps_rb[:, 0:21], ones_row, rnl[:, 0:21], start=True,
                         stop=True)
        gfull = work.tile([H, 21], F32, tag="gfull")
        v.scalar_tensor_tensor(gfull, lhs_n[:, 0:21], lh_sb,
                               ps_rb[0:H, 0:21], op0=OP.mult, op1=OP.mult)
        for t in range(2):
            ps_f = ps_pool.tile([128, 512], F32, tag="ps_b", bufs=4)
            nc.tensor.matmul(ps_f[:, 0:21],
                             sa["xTm"][0:H, t * 128:(t + 1) * 128], gfull,
                             start=True, stop=True)
            v.tensor_tensor(out_sb[A][:, t, 2:23], ps_f[:, 0:21],
                            sa["comb_full"][:, t, :], op=OP.mult)

    # ---------------- maxpool means ----------------
    for A, B in (("p", "h"), ("h", "p")):
        sa, sb = sides[A], sides[B]
        ps_s = ps_pool.tile([128, 512], F32, tag="ps_b", bufs=4)
        for jt in range(2):
            nc.tensor.matmul(ps_s[0:H, 0:P], sb["xm_s"][:, jt, :],
                             sb["rnT"][:, jt, GRP_MP:GRP_MP + P],
                             start=(jt == 0), stop=(jt == 1))
        g_mp = work.tile([H, P], F32, tag="g_mp")
        v.scalar_tensor_tensor(g_mp, ps_s[0:H, 0:P], sb["invlen_col"][0:H, :],
                               lhs_n[:, GRP_MP:GRP_MP + P],
                               op0=OP.mult, op1=OP.mult)
        for t in range(2):
            ps_m = ps_pool.tile([128, 512], F32, tag="ps_b", bufs=4)
            nc.tensor.matmul(ps_m[:, 0:P],
                             sa["xTm"][0:H, t * 128:(t + 1) * 128], g_mp,
                             start=True, stop=True)
            v.tensor_tensor(out_sb[A][:, t, 43:63], ps_m[:, 0:P],
                            sa["rnT"][:, t, GRP_MP:GRP_MP + P], op=OP.mult)

    # ---- per-side heavy block: mp-max part1 + att_max + per-t tails -----
    # Emission order per side: (1) bc broadcast DMAs (in flight early),
    # (2) mp-max rhs builds + PE matmuls + scalar evacs (PE/scalar pipeline
    # under the upcoming pure-DVE phase), (3) t-major att_max products +
    # max trees on DVE, then per-t: mp-max bf16 tree, out-muls and the
    # mv-ma tail chain, and that t-slice's output DMA.
    rnp_pool = ctx.enter_context(tc.tile_pool(name="rnp", bufs=10))
    for A, B in (("p", "h"), ("h", "p")):
        sa, sb = sides[A], sides[B]
        d = 0 if A == "p" else 1
        am = persist.tile([128, 2, H], F32, tag=f"att_max_{A}")
        d_rep = sb["d_rep"]

        # (1) bc broadcast DMAs
        bcs = []
        for c in range(NCH):
            bc_c = bc_pool.tile([128, HC, S], BF16, tag=f"bc_{c}",
                                name=f"bc_{c}_{A}")
            dma(out=bc_c, in_=d_rep[c * HC:(c + 1) * HC, :]
                .partition_broadcast(128))
            bcs.append(bc_c)

        # (2) mp-max part1: rnpair prefetch + rhs builds + matmuls + evacs
        maxraw = persist.tile([128, 2, P], F32, tag=f"maxraw_{A}")
        rhs_pair = [persist.tile([101, 2, S], BF16, tag=f"rhsp{i}_{A}",
                                 name=f"rhsp{i}_{A}")
                    for i in range(2)]
        for i in range(2):
            for kk in range(2):
                sc.activation(rhs_pair[i][96:101, kk, :],
                              sb["m_b"][96:101, :], AF.Copy,
                              bias=MIN_VAL, scale=-MIN_VAL)
        mp16 = [mp_pool.tile([128, P, S], BF16, tag="mp16",
                             name=f"mp16_{A}_{t}") for t in range(2)]
        NPI = P // 2 if "mpmax" not in ablate else 0
        rnps = []
        for pi in range(NPI):
            rnpair = rnp_pool.tile([128, 2, S], BF16, tag="rnpair",
                                   name=f"rnpair_{A}_{pi}")
            dma(out=rnpair,
                in_=sb["d_rn"][GRP_MP + 2 * pi:GRP_MP + 2 * pi + 2,
                               :].partition_broadcast(128))
            rnps.append(rnpair)
        for pi in range(NPI):
            rp = rhs_pair[pi % 2]
            for kk in range(2):
                v.scalar_tensor_tensor(
                    rp[0:H, kk, :], sb["xTm"][0:H, :],
                    lhs_n[:, GRP_MP + 2 * pi + kk:GRP_MP + 2 * pi + kk + 1],
                    rnps[pi][0:H, kk, :], op0=OP.mult, op1=OP.mult)
            for t in range(2):
                ps_x = ps_pool.tile([128, 512], F32, tag="ps_a")
                nc.tensor.matmul(ps_x[:, 0:2 * S],
                                 sa["xTm16"][:, t * 128:(t + 1) * 128],
                                 rp[:].rearrange("p a b -> p (a b)"),
                                 start=True, stop=True)
                sc.activation(mp16[t][:, 2 * pi:2 * pi + 2, :]
                              .rearrange("p a b -> p (a b)"),
                              ps_x[:, 0:2 * S], AF.Copy, bias=0.0, scale=1.0)

        # (3) t-major att_max products + trees, then per-t tails
        amT = persist.tile([H, S], F32, tag=f"amT_{A}")
        for t in range(2):
            if "attmax" not in ablate:
                for c in range(NCH):
                    a_bc = (att_sb[A][:, t, 0:S].unsqueeze(1)
                            .to_broadcast((128, HC, S)))
                    prod = tree_pool.tile([128, HC, S], BF16, tag="prod",
                                          name=f"prod_{A}_{c}_{t}")
                    v.tensor_tensor(prod, a_bc, bcs[c], op=OP.mult)
                    t1 = tree_pool.tile([128, HC, 128], BF16, tag="t1",
                                        name=f"t1_{A}_{c}_{t}")
                    t2 = tree_pool.tile([128, HC, 64], BF16, tag="t2",
                                        name=f"t2_{A}_{c}_{t}")
                    v.tensor_tensor(t1, prod[:, :, 0:128],
                                    prod[:, :, 128:256], op=OP.max)
                    v.tensor_tensor(t2, t1[:, :, 0:64], t1[:, :, 64:128],
                                    op=OP.max)
                    v.tensor_tensor(t1[:, :, 0:32], t2[:, :, 0:32],
                                    t2[:, :, 32:64], op=OP.max)
                    v.tensor_tensor(t2[:, :, 0:16], t1[:, :, 0:16],
                                    t1[:, :, 16:32], op=OP.max)
                    v.tensor_tensor(t1[:, :, 0:8], t2[:, :, 0:8],
                                    t2[:, :, 8:16], op=OP.max)
                    v.tensor_tensor(t2[:, :, 0:4], t1[:, :, 0:4],
                                    t1[:, :, 4:8], op=OP.max)
                    v.tensor_tensor(t1[:, :, 0:2], t2[:, :, 0:2],
                                    t2[:, :, 2:4], op=OP.max)
                    v.tensor_tensor(am[:, t, c * HC:(c + 1) * HC],
                                    t1[:, :, 0:1], t1[:, :, 1:2], op=OP.max)

            # mp-max bf16 tree for this t
            if "mpmax" not in ablate:
                m16 = mp16[t]
                u1 = tree_pool.tile([128, P, 128], BF16, tag="t1",
                                    name=f"u1_{A}_{t}")
                u2 = tree_pool.tile([128, P, 64], BF16, tag="t2",
                                    name=f"u2_{A}_{t}")
                v.tensor_tensor(u1, m16[:, :, 0:128], m16[:, :, 128:256],
                                op=OP.max)
                v.tensor_tensor(u2, u1[:, :, 0:64], u1[:, :, 64:128],
                                op=OP.max)
                v.tensor_tensor(u1[:, :, 0:32], u2[:, :, 0:32],
                                u2[:, :, 32:64], op=OP.max)
                v.tensor_tensor(u2[:, :, 0:16], u1[:, :, 0:16],
                                u1[:, :, 16:32], op=OP.max)
                v.tensor_tensor(u1[:, :, 0:8], u2[:, :, 0:8], u2[:, :, 8:16],
                                op=OP.max)
                v.tensor_tensor(u2[:, :, 0:4], u1[:, :, 0:4], u1[:, :, 4:8],
                                op=OP.max)
                v.tensor_tensor(u1[:, :, 0:2], u2[:, :, 0:2], u2[:, :, 2:4],
                                op=OP.max)
                v.tensor_tensor(maxraw[:, t, :], u1[:, :, 0:1], u1[:, :, 1:2],
                                op=OP.max)
            v.tensor_tensor(out_sb[A][:, t, 23:43], maxraw[:, t, :],
                            sa["rnT"][:, t, GRP_MP:GRP_MP + P], op=OP.mult)

            # mv-ma tail for this t
            if "mv" not in ablate:
                ps_at = ps_pool.tile([128, 512], F32, tag="ps_b", bufs=4)
                nc.tensor.transpose(ps_at[0:H, 0:128], am[:, t, :], ident)
                sc.activation(amT[:, t * 128:(t + 1) * 128],
                              ps_at[0:H, 0:128], AF.Copy, bias=0.0,
                              scale=1.0)
                cmA = work.tile([H, 128], F32, tag="cmA")
                v.tensor_mul(cmA, sa["xTm"][0:H, t * 128:(t + 1) * 128],
                             amT[:, t * 128:(t + 1) * 128])
                m2A = work.tile([H, 128], F32, tag="m2A")
                sc.square(m2A, amT[:, t * 128:(t + 1) * 128])
                ps_num = ps_pool.tile([128, 512], F32, tag="ps_b", bufs=4)
                nc.tensor.matmul(ps_num[:, 0:21], cmA, g_ma, start=True,
                                 stop=True)
                ps_msq = ps_pool.tile([128, 512], F32, tag="ps_b", bufs=4)
                nc.tensor.matmul(ps_msq[:, 0:21], m2A, g_ma, start=True,
                                 stop=True)
                nm = work.tile([128, 21], F32, tag="nm")
                sc.activation(nm, ps_msq[:, 0:21], AF.Sqrt, bias=cl_col,
                              scale=1.0)
                rnm = work.tile([128, 21], F32, tag="rnm")
                v.reciprocal(rnm, nm)
                t21 = work.tile([128, 21], F32, tag="t21")
                v.tensor_tensor(t21, rnm, sa["comb_ma"][:, t, :], op=OP.mult)
                v.tensor_tensor(out_sb[A][:, t, 84:105], ps_num[:, 0:21],
                                t21, op=OP.mult)

            # this t-slice of the output is complete — ship it
            dma(out=d_out[d, t * 128:(t + 1) * 128, :],
                in_=out_sb[A][:, t, :])

    ctx.close()


_NC = None


def _get_nc():
    global _NC
    if _NC is None:
        _NC = _build(bass.Bass())
    return _NC


def kernel(context_p, mask_p, context_h, mask_h, w_full, w_maxpool, w_att,
           w_maxatt):
    B = context_p.shape[0]
    assert B == N_CORES
    wT = np.ascontiguousarray(
        np.concatenate([w_full, w_maxpool, w_att, w_maxatt], 0).T)  # (H, 80)
    in_maps = []
    for b in range(B):
        in_maps.append({
            "cpT": np.ascontiguousarray(context_p[b].T),
            "chT": np.ascontiguousarray(context_h[b].T),
            "cps": np.ascontiguousarray(context_p[b]),
            "chs": np.ascontiguousarray(context_h[b]),
            "mp": np.ascontiguousarray(mask_p[b][None, :]),
            "mh": np.ascontiguousarray(mask_h[b][None, :]),
            "wT": wT,
        })
    nc = _get_nc()
    res = run_bass_kernel_spmd(nc, in_maps, core_ids=list(range(N_CORES)),
                               trace=bool(int(os.environ.get("KTRACE", "0"))))
    out = np.stack([res.results[b]["out"] for b in range(B)], 1)
    if os.environ.get("KTRACE") and res.exec_time_ns is not None:
        print(f"HW exec time: {res.exec_time_ns} ns")
    kernel._last = res
    return out

